# revision 5
# baseline (speedup 1.0000x reference)
"""AxonLIFNode forward on 8 Trainium2 NeuronCores.

Reference recurrence (per element, sequential over T):
    mem   = mem + (x_t + V_RESET - mem) / TAU        # V_RESET=0, TAU=2
    spike = (mem - V_TH > 0)                         # V_TH=1, {0.0, 1.0}
    mem   = (1 - spike) * mem + V_RESET * spike      # reset to 0 on spike
    out_i = out_i * sigmoid(w) + spike               # axon current (w=0 -> 0.5)
    outputs: (spike, out_i), both [B, T, N] f32

Strategy: data-parallel over the batch axis (B=64 -> 8 per core). Per core the
32768 independent series are laid out as 128 partitions x 256 free elements;
the T=64 loop runs as a serial chain of one fused custom-DVE instruction per
timestep computing the pre-reset membrane m1_t from (x_t, m1_{t-1}):

    prev = m1 * (m1 <= 1)            # reset of the *previous* pre-reset mem
    m1'  = prev + (x - prev) * 0.5

which is bit-exact vs. the reference ordering (each ALU stage is one IEEE f32
rounding; *0.5 == /2 exactly). Spikes fall out as is_gt(m1, 1) and the axon
current via the stock AFFINE_THEN_ADD fused op. X streams in / outputs stream
out in groups of G timesteps, double buffered, overlapping DMA with the DVE
chain.
"""

import numpy as np

import concourse.bacc as bacc
import concourse.mybir as mybir
import concourse.dve_ops as dve_ops
from concourse.dve_ops import DveOp
from concourse.dve_spec import Spec, Src0, Src1, C0, C1, lower
from concourse.dve_uop import DveOpSpec
from concourse.tile import TileContext
from concourse.bass_utils import run_bass_kernel_spmd

# Problem shape (hardcoded per harness contract).
B, T, N = 64, 64, 4096
CORES = 8
BS = B // CORES          # batches per core
P = 128                  # SBUF partitions
J = 16                   # n-chunks per batch: BS * J == P
F = N // J               # free elements per partition per timestep (256)
G = 8                    # timesteps per DMA group

_LIF_OP_NAME = "LIF_M1_ANT"


def _register_lif_op() -> DveOp:
    """Register the fused LIF membrane-update op in the custom-DVE registry.

    out = prev + (Src0 - prev) * C0,  prev = Src1 * (Src1 <= C1)
    Src0 = x_t, Src1 = m1_{t-1}, C0 = 1/TAU, C1 = V_TH.
    """
    for op in dve_ops.OPS:
        if op.name == _LIF_OP_NAME:
            return op

    keep = Src1 <= C1
    prev = Src1 * keep
    body = prev + (Src0 - prev) * C0

    def _ref(in0, in1, s0, s1, imm2):
        p = (in1 * (in1 <= s1)).astype(np.float32)
        return (p + (in0 - p) * np.float32(s0)).astype(np.float32)

    spec = Spec(body=body, reference=_ref)
    row = dve_ops._CUSTOM_DVE_ROW_BASE + len(dve_ops.OPS)
    assert row < 0x20, "custom-DVE opcode rows exhausted"
    shas = {}
    for ver in ("v3", "v4"):
        uops = lower(spec, ver=ver)
        shas[ver] = DveOpSpec(
            name=_LIF_OP_NAME, opcode=row, uops=uops, rd1_en=True
        ).sha(ver)
    op = DveOp(_LIF_OP_NAME, spec, subdim=False, uops_sha=shas)
    dve_ops._SUB_OPCODE_FOR_NAME[op.name] = row
    dve_ops.OPS.append(op)
    dve_ops.CUSTOM_DVE_SPECS[op.name] = spec
    return op


_nc_cache: dict = {}


def _build(inv_tau: float):
    """Trace + compile the per-core Bass program (SPMD: same NEFF, 8 cores)."""
    key = float(inv_tau)
    if key in _nc_cache:
        return _nc_cache[key]

    lif_op = _register_lif_op()
    f32 = mybir.dt.float32

    nc = bacc.Bacc(
        "TRN2",
        target_bir_lowering=False,
        debug=False,
        enable_asserts=False,
        num_devices=CORES,
    )
    # Host pre-transposes each core's shard to [(b j) = 128, T, F] contiguous,
    # so every DMA is a 3-dim AP with an 8 KB contiguous run per partition.
    x_r = nc.dram_tensor("x", [P, T, F], f32, kind="ExternalInput").ap()
    spk_r = nc.dram_tensor("spk", [P, T, F], f32, kind="ExternalOutput").ap()
    oi_r = nc.dram_tensor("oi", [P, T, F], f32, kind="ExternalOutput").ap()

    with TileContext(nc) as tc:
        with (
            tc.tile_pool(name="const", bufs=1) as cpool,
            tc.tile_pool(name="xin", bufs=3) as xpool,
            tc.tile_pool(name="sout", bufs=2) as spool,
            tc.tile_pool(name="oout", bufs=2) as opool,
            tc.tile_pool(name="mstate", bufs=4) as mpool,
        ):
            m_init = cpool.tile([P, F], f32)
            nc.vector.memset(m_init[:], 0.0)
            m_prev = m_init[:]
            oi_prev = None  # first step: out_i = 0 * inv_tau + spike = spike

            for g in range(T // G):
                x_t = xpool.tile([P, G, F], f32)
                nc.sync.dma_start(out=x_t[:], in_=x_r[:, g * G : (g + 1) * G, :])
                s_t = spool.tile([P, G, F], f32)
                o_t = opool.tile([P, G, F], f32)
                for k in range(G):
                    m_new = mpool.tile([P, F], f32)
                    nc.vector._custom_dve(
                        lif_op,
                        out=m_new[:],
                        in0=x_t[:, k, :],
                        in1=m_prev,
                        s0=0.5,      # 1/TAU
                        s1=1.0,      # V_TH
                    )
                    nc.vector.tensor_scalar(
                        out=s_t[:, k, :],
                        in0=m_new[:],
                        scalar1=1.0,
                        scalar2=None,
                        op0=mybir.AluOpType.is_gt,
                    )
                    if oi_prev is None:
                        nc.vector.tensor_copy(out=o_t[:, k, :], in_=s_t[:, k, :])
                    else:
                        # o = (oi_prev * inv_tau + 0) + spike
                        nc.vector.affine_then_add(
                            out=o_t[:, k, :],
                            in0=oi_prev,
                            in1=s_t[:, k, :],
                            scale=inv_tau,
                            bias=0.0,
                        )
                    m_prev = m_new[:]
                    oi_prev = o_t[:, k, :]
                nc.sync.dma_start(out=spk_r[:, g * G : (g + 1) * G, :], in_=s_t[:])
                nc.sync.dma_start(out=oi_r[:, g * G : (g + 1) * G, :], in_=o_t[:])

    nc.compile()
    _nc_cache[key] = nc
    return nc


def _shard(X: np.ndarray) -> list[np.ndarray]:
    """[B, T, N] -> per-core [(b j) = 128, T, F] contiguous."""
    Xt = np.ascontiguousarray(
        X.reshape(B, T, J, F).transpose(0, 2, 1, 3)
    )  # [B, J, T, F]
    return [
        Xt[c * BS : (c + 1) * BS].reshape(P, T, F) for c in range(CORES)
    ]


def _unshard(parts: list[np.ndarray]) -> np.ndarray:
    """per-core [(b j), T, F] -> [B, T, N]."""
    full = np.stack(parts).reshape(B, J, T, F)
    return np.ascontiguousarray(full.transpose(0, 2, 1, 3)).reshape(B, T, N)


def _run(X: np.ndarray, w: np.ndarray, **spmd_kwargs):
    X = np.asarray(X, dtype=np.float32)
    inv_tau = float(1.0 / (1.0 + np.exp(-np.float64(np.asarray(w).item()))))
    nc = _build(inv_tau)
    in_maps = [{"x": xs} for xs in _shard(X)]
    res = run_bass_kernel_spmd(nc, in_maps, core_ids=list(range(CORES)), **spmd_kwargs)
    spikes = _unshard([res.results[c]["spk"] for c in range(CORES)])
    i_pot = _unshard([res.results[c]["oi"] for c in range(CORES)])
    return (spikes, i_pot), res


def kernel(X: np.ndarray, w: np.ndarray):
    out, _ = _run(X, w)
    return out
